# revision 2
# baseline (speedup 1.0000x reference)
"""Causal single-head attention (B=4, S=2048, D=1024) on 8 trn2 NeuronCores.

Sharding: 8 cores = 4 batches x 2 sequence-shards. Queries split into eight
256-row chunks per batch; core (b, p) handles global chunks {2c+p}; local
chunk c attends to nkt = 4(c+1) key tiles of 128, the 512-key diagonal block
masked multiplicatively per-core after exp.

Algebraic restructure — no K or V projection, no duplicated work, and no
cross-core communication:
  scores = (x Wq^T)(x Wk^T)^T / sqrt(D) = x M x^T,  M = Wq^T Wk / sqrt(D)
  output = softmax(scores) (x Wv^T + bv) = ((p x) Wv^T) / den + bv
M is precomputed on the host, so the kernel does: q' = x_q M (per-query),
scores against RAW x^T (stationary is an input tile), pv = p x (attention-
weighted input rows), then a final per-query projection o = pv Wv^T. The
K/V projections over the full 2048 keys — previously duplicated across the
two cores of a batch — are replaced by per-query projections of size
SQ x D x D, eliminating ~40% of the baseline's PE work per core.

Bias handling is exact: the per-query score bias (x_q Wq^T) bk is softmax-
invariant and dropped; the per-key bias bq (Wk x_k^T) is a host-computed
per-key scalar fed as the exp's per-partition bias; bv is added after the
normalizing divide. All matmuls bf16 with fp32 PSUM accumulation (fp8
tested: fails the 2e-2 gate — attention output is a weighted mean of
near-zero-mean values, so quantization error does not average down).
Output is written bf16 and transposed [E, SQ]; the host transposes back.

PSUM tiles are bank-granular (2KB), so all attention psums pack pairs into
[128, 512] banks: scores hold a kt-pair (two accumulation groups), pv holds
a dt-pair, o1 holds an et-pair; the q'-projection reuses the o1 ring.
"""

import numpy as np
import ml_dtypes
from contextlib import ExitStack

import concourse.bacc as bacc
import concourse.bass as bass
import concourse.mybir as mybir
import concourse.tile as tile
from concourse import bass_utils

bf16 = ml_dtypes.bfloat16
f32 = np.float32

B, S, D = 4, 2048, 1024
E = D
N_CORES = 8
QCH = 256            # query chunk rows
NCH = 4              # chunks per core
SQ = QCH * NCH       # 1024 query rows per core
DT = D // 128        # 8 d-tiles
ET = E // 128        # 8 e-tiles
KT = S // 128        # 16 key tiles
NKT = [4, 8, 12, 16]  # key tiles per local chunk

_CACHE = {}


def _build():
    nc = bacc.Bacc("TRN2")
    dt_bf16 = mybir.dt.bfloat16
    dt_f32 = mybir.dt.float32
    ADD = mybir.AluOpType.add

    xtq = nc.dram_tensor("xtq", [D, SQ], dt_bf16, kind="ExternalInput")
    xkt = nc.dram_tensor("xkt", [D, S], dt_bf16, kind="ExternalInput")
    xnt = nc.dram_tensor("xnt", [S, D], dt_bf16, kind="ExternalInput")
    mm = nc.dram_tensor("mm", [D, D], dt_bf16, kind="ExternalInput")
    wvt = nc.dram_tensor("wvt", [D, E], dt_bf16, kind="ExternalInput")
    wkb = nc.dram_tensor("wkb", [128, KT], dt_f32, kind="ExternalInput")
    bvs = nc.dram_tensor("bvs", [128, ET], dt_f32, kind="ExternalInput")
    maskt = nc.dram_tensor("maskt", [512, QCH], dt_bf16, kind="ExternalInput")
    o = nc.dram_tensor("o", [E, SQ], dt_bf16, kind="ExternalOutput")

    Ident = mybir.ActivationFunctionType.Identity
    Exp = mybir.ActivationFunctionType.Exp

    with ExitStack() as ctx:
        tc = ctx.enter_context(tile.TileContext(nc))
        persist = ctx.enter_context(tc.tile_pool(name="persist", bufs=1))

        mmt = [persist.tile([128, D], dt_bf16, tag=f"mm{i}", name=f"mm{i}") for i in range(DT)]
        wv = [persist.tile([128, E], dt_bf16, tag=f"wv{i}", name=f"wv{i}") for i in range(DT)]
        xq = [persist.tile([128, SQ], dt_bf16, tag=f"xq{i}", name=f"xq{i}") for i in range(DT)]
        xk = [persist.tile([128, S], dt_bf16, tag=f"xk{i}", name=f"xk{i}") for i in range(DT)]
        xn = [persist.tile([128, D], dt_bf16, tag=f"xn{i}", name=f"xn{i}") for i in range(KT)]
        qt = [persist.tile([128, SQ], dt_bf16, tag=f"qt{i}", name=f"qt{i}") for i in range(ET)]
        msk = [persist.tile([128, QCH], dt_bf16, tag=f"m{i}", name=f"m{i}") for i in range(4)]
        wkb_sb = persist.tile([128, KT], dt_f32, tag="wkb")
        bvs_sb = persist.tile([128, ET], dt_f32, tag="bvs")
        onespad = persist.tile([128, 128], dt_bf16, tag="onespad")
        warm = persist.tile([128, 512], dt_bf16, tag="warm")

        nc.vector.memset(onespad[:], 0.0)
        nc.vector.memset(onespad[:, 0:1], 1.0)
        nc.vector.memset(warm[:], 0.0)

        # 8 PSUM banks: pss 2 + pvp 2 + pso 3 + psd 1
        pss = ctx.enter_context(tc.tile_pool(name="pss", bufs=2, space="PSUM"))
        pvp = ctx.enter_context(tc.tile_pool(name="pvp", bufs=1, space="PSUM"))
        pso = ctx.enter_context(tc.tile_pool(name="pso", bufs=3, space="PSUM"))
        psd = ctx.enter_context(tc.tile_pool(name="psd", bufs=1, space="PSUM"))
        sb = ctx.enter_context(tc.tile_pool(name="sb", bufs=1))

        # ---- HAM warm-up while the first DMAs land ----
        wps = pso.tile([128, 512], dt_f32, tag="o1", name="pswarm")
        for _ in range(12):
            nc.tensor.matmul(wps[:], warm[:, 0:128], warm[:],
                             start=True, stop=True)

        # ---- input DMAs: q'-projection inputs first, then x^T / x-natural
        # for attention, then Wv^T; constants just behind the lead tiles ----
        for i in range(DT):
            nc.sync.dma_start(out=mmt[i][:, 0:512],
                              in_=mm.ap()[i * 128:(i + 1) * 128, 0:512])
            nc.sync.dma_start(out=xq[i][:, 0:512],
                              in_=xtq.ap()[i * 128:(i + 1) * 128, 0:512])
        for i in range(DT):
            nc.sync.dma_start(out=xq[i][:, 512:1024],
                              in_=xtq.ap()[i * 128:(i + 1) * 128, 512:1024])
        for i in range(DT):
            nc.sync.dma_start(out=mmt[i][:, 512:1024],
                              in_=mm.ap()[i * 128:(i + 1) * 128, 512:1024])
        nc.sync.dma_start(out=wkb_sb[:], in_=wkb.ap())
        for i in range(4):
            nc.sync.dma_start(out=msk[i][:],
                              in_=maskt.ap()[i * 128:(i + 1) * 128, :])
        # x^T columns for early key tiles first
        for h in range(2):
            for i in range(DT):
                nc.sync.dma_start(out=xk[i][:, h * 1024:(h + 1) * 1024],
                                  in_=xkt.ap()[i * 128:(i + 1) * 128,
                                               h * 1024:(h + 1) * 1024])
        for kt in range(KT):
            nc.sync.dma_start(out=xn[kt][:], in_=xnt.ap()[kt * 128:(kt + 1) * 128, :])
        nc.sync.dma_start(out=bvs_sb[:], in_=bvs.ap())
        for i in range(DT):
            nc.sync.dma_start(out=wv[i][:], in_=wvt.ap()[i * 128:(i + 1) * 128, :])

        # ---- q'^T projection: qt[d', sq] = sum_d M[d, d'] x_q^T[d, sq] ----
        for et in range(ET):
            pss_ = [pso.tile([128, 512], dt_f32, tag="o1", name=f"psq{et}_{c}")
                    for c in range(SQ // 512)]
            for i in range(DT):
                for c in range(SQ // 512):
                    nc.tensor.matmul(
                        pss_[c][:],
                        mmt[i][:, et * 128:(et + 1) * 128],
                        xq[i][:, c * 512:(c + 1) * 512],
                        start=(i == 0), stop=(i == DT - 1),
                    )
            for c in range(SQ // 512):
                if c % 2 == 0:
                    nc.scalar.activation(qt[et][:, c * 512:(c + 1) * 512],
                                         pss_[c][:], Ident)
                else:
                    nc.vector.tensor_copy(qt[et][:, c * 512:(c + 1) * 512],
                                          pss_[c][:])

        # ---- attention chunks ----
        for li in range(NCH):
            nkt = NKT[li]
            qc = li * QCH
            pts = []          # one [128, 512] tile per kt-PAIR
            for jk in range(nkt // 2):
                sps = pss.tile([128, 512], dt_f32, tag="st", name="st")
                pt = sb.tile([128, 512], dt_bf16, tag="pt", bufs=12, name="pt")
                for i2 in range(2):
                    kt = 2 * jk + i2
                    hs = slice(i2 * QCH, (i2 + 1) * QCH)
                    for i in range(DT):
                        nc.tensor.matmul(
                            sps[:, hs],
                            xk[i][:, kt * 128:(kt + 1) * 128],
                            qt[i][:, qc:qc + QCH],
                            start=(i == 0), stop=(i == DT - 1),
                        )
                    nc.scalar.activation(pt[:, hs], sps[:, hs], Exp,
                                         bias=wkb_sb[:, kt:kt + 1])
                    if kt >= nkt - 4:
                        nc.vector.tensor_mul(pt[:, hs], pt[:, hs],
                                             msk[kt - (nkt - 4)][:])
                pts.append(pt)

            def pslice(kt):
                return pts[kt // 2][:, (kt % 2) * QCH:(kt % 2 + 1) * QCH]

            # denominator: den[q] = sum_k p[k, q] lands in row 0 of a
            # [128, 256] psum (ones-padded stationary streams at full rate;
            # a [1, 256]-out matmul measures ~1.6x slower)
            dps = psd.tile([128, QCH], dt_f32, tag="d", name="d")
            for kt in range(nkt):
                nc.tensor.matmul(dps[:], onespad[:], pslice(kt),
                                 start=(kt == 0), stop=(kt == nkt - 1))
            den_r = sb.tile([1, QCH], dt_f32, tag="denr", bufs=2, name="denr")
            nc.vector.reciprocal(den_r[:], dps[0:1, :])
            den_bc = sb.tile([128, QCH], dt_f32, tag="denbc", bufs=2,
                             name="denbc")
            nc.gpsimd.partition_broadcast(den_bc[:], den_r[:])

            # pv^T[d, q] = sum_k x[k, d] p[k, q], two dt-halves x two dt-pairs
            pv_sb = []
            for hf in range(2):
                pvp_t = [pvp.tile([128, 512], dt_f32, tag=f"pv{j}", name=f"pv{j}")
                         for j in range(2)]
                # j outer: a bank's two accumulation groups must run
                # back-to-back, not interleaved — start=True pending-zeroes
                # the whole 2KB bank region, clobbering an in-flight
                # sibling group's partial sums
                for j in range(4):
                    dt_i = 4 * hf + j
                    for kt in range(nkt):
                        nc.tensor.matmul(
                            pvp_t[j // 2][:, (j % 2) * QCH:(j % 2 + 1) * QCH],
                            xn[kt][:, dt_i * 128:(dt_i + 1) * 128],
                            pslice(kt),
                            start=(kt == 0), stop=(kt == nkt - 1),
                        )
                for j in range(4):
                    pv = sb.tile([128, QCH], dt_bf16, tag="pvsb", bufs=10,
                                 name="pvsb")
                    src = pvp_t[j // 2][:, (j % 2) * QCH:(j % 2 + 1) * QCH]
                    if j % 2 == 0:
                        nc.scalar.activation(pv[:], src, Ident)
                    else:
                        nc.vector.tensor_copy(pv[:], src)
                    pv_sb.append(pv)

            # o1^T[e, q] = sum_d Wv^T[d, e] pv^T[d, q]; divide + bias + out
            for ep in range(ET // 2):
                o1 = pso.tile([128, 512], dt_f32, tag="o1", name="o1")
                for e2 in range(2):
                    et = 2 * ep + e2
                    hs = slice(e2 * QCH, (e2 + 1) * QCH)
                    for i in range(DT):
                        nc.tensor.matmul(
                            o1[:, hs],
                            wv[i][:, et * 128:(et + 1) * 128],
                            pv_sb[i][:],
                            start=(i == 0), stop=(i == DT - 1),
                        )
                    o_sb = sb.tile([128, QCH], dt_bf16, tag="osb", bufs=4,
                                   name="osb")
                    nc.vector.tensor_mul(o_sb[:], o1[:, hs], den_bc[:])
                    nc.vector.tensor_scalar(out=o_sb[:], in0=o_sb[:],
                                            scalar1=bvs_sb[:, et:et + 1],
                                            scalar2=None, op0=ADD)
                    nc.sync.dma_start(
                        out=o.ap()[et * 128:(et + 1) * 128, qc:qc + QCH],
                        in_=o_sb[:],
                    )

    nc.compile()
    return nc


def _host_shard(inputs, Wq, bq, Wk, bk, Wv, bv):
    """Build the 8 per-core input maps."""
    scale = np.float32(np.sqrt(D))
    M = (Wq.T.astype(np.float64) @ Wk.astype(np.float64) / scale).astype(f32)
    mmv = np.ascontiguousarray(M).astype(bf16)
    wvt = np.ascontiguousarray(Wv.T).astype(bf16)
    bvs = np.ascontiguousarray(bv.reshape(ET, 128).T).astype(f32)
    hk = (Wk.T @ bq) / scale                 # [D]; per-key score bias source

    kk = np.arange(512)[:, None]
    qq = np.arange(QCH)[None, :]
    mask_p0 = np.where(kk < 256, (kk <= qq), False).astype(bf16)
    mask_p1 = np.where(kk < 256, True, (kk - 256) <= qq).astype(bf16)
    masks = [mask_p0, mask_p1]

    in_maps = []
    for core in range(N_CORES):
        b, p = divmod(core, 2)
        xb = inputs[b]                       # [S, D] fp32
        rows = np.concatenate(
            [xb[QCH * (2 * c + p): QCH * (2 * c + p) + QCH] for c in range(NCH)],
            axis=0,
        )                                    # [SQ, D]
        wkbv = (xb @ hk).astype(f32)         # [S] per-key exp bias
        in_maps.append({
            "xtq": np.ascontiguousarray(rows.T).astype(bf16),
            "xkt": np.ascontiguousarray(xb.T).astype(bf16),
            "xnt": np.ascontiguousarray(xb).astype(bf16),
            "mm": mmv, "wvt": wvt,
            "wkb": np.ascontiguousarray(wkbv.reshape(KT, 128).T),
            "bvs": bvs,
            "maskt": masks[p],
        })
    return in_maps


def _assemble(results, dtype):
    out = np.empty((B, S, E), dtype=dtype)
    for core in range(N_CORES):
        b, p = divmod(core, 2)
        oc = results[core]["o"].astype(dtype)   # [E, SQ]
        for c in range(NCH):
            g = 2 * c + p
            out[b, QCH * g: QCH * (g + 1)] = oc[:, QCH * c: QCH * (c + 1)].T
    return out


def kernel(inputs, Wq, bq, Wk, bk, Wv, bv):
    inputs = np.asarray(inputs, dtype=f32)
    Wq, bq = np.asarray(Wq, dtype=f32), np.asarray(bq, dtype=f32)
    Wk, bk = np.asarray(Wk, dtype=f32), np.asarray(bk, dtype=f32)
    Wv, bv = np.asarray(Wv, dtype=f32), np.asarray(bv, dtype=f32)

    if "nc" not in _CACHE:
        _CACHE["nc"] = _build()
    nc = _CACHE["nc"]

    in_maps = _host_shard(inputs, Wq, bq, Wk, bk, Wv, bv)
    res = bass_utils.run_bass_kernel_spmd(nc, in_maps, core_ids=list(range(N_CORES)))
    return _assemble(res.results, f32)
